# revision 3
# baseline (speedup 1.0000x reference)
"""Bipartite GCN message-passing kernel for 8 Trainium2 NeuronCores.

Math (reference): rst = deg_in^-1/2 * segsum_dst( (node_f @ W_side) * deg_out^-1/2 [src] )
Refactor (projection is linear, graph strictly bipartite):
    rst[d] = ( sum_{e->d} c_e * f_raw[src_e] ) @ W_side(d),
    c_e = deg_out[src]^-1/2 * deg_in[dst]^-1/2

Division of labor:
  HOST (layout / index math only — no feature arithmetic):
    degree counts, per-core dst dealing, canonical chunk schedule, and a
    bf16 edge-major re-layout of the raw feature rows (M tiles = f[src_e]
    placed at its schedule position) plus compact scatter blocks S holding
    c_e, merged into one stream per window.  This replaces the v1
    device-side dma_gather, whose GPSIMD descriptor generation (~8 ns/edge,
    serial on the Q7s) was a hard 1.6 ms floor.
  DEVICE (all feature FLOPs):
    per window: stream the merged M|S tile sequentially at DMA line rate,
    aggregate PSUM[feat, dst_slot] += M_chunk[128e,128f].T @ S_chunk[128e,cols]
    (bf16 matmuls, fp32 accumulate), then project with W_side (fp32) and
    stream out the [128, slots] feature-major result window by window.

Sharding: dst nodes dealt round-robin (degree-sorted) to 8 cores ->
identical compile-time schedule per core (SPMD), no collectives.
"""
import sys
import os

for _p in ("/opt/trn_rl_repo",):
    if _p not in sys.path and os.path.isdir(_p):
        sys.path.insert(0, _p)

import numpy as np
import ml_dtypes

BF16 = ml_dtypes.bfloat16

N_U = 50000
N_V = 50000
N = N_U + N_V
D = 128
E = 1600000
N_CORES = 8
WIN = 512             # dst slots per PSUM window
P = 128
NBUF = 3              # input stream buffers


# ----------------------------------------------------------------- host layout
def _build_layout(src, dst, cout, cin, u_bf, v_bf):
    """Canonical schedule + per-core merged M|S stream data.

    Returns (wlist, totals, per_core). wlist is the compile-time window
    list (identical across cores); per_core holds the merged ms array +
    the slot -> global dst id mapping.
    """
    wlist = []
    per_core_ms = [[] for _ in range(N_CORES)]
    per_core_dsts = [[] for _ in range(N_CORES)]

    ms_base = 0           # global ms column counter
    slot_base = 0         # global output slot counter

    for phase in range(2):
        if phase == 0:    # dsts are v-nodes, sources u-side
            mask = dst >= N_U
            d_local = dst[mask] - N_U
            s_local = src[mask]
            feats = u_bf
            dst_base = N_U
            src_base = 0
        else:             # dsts are u-nodes, sources v-side
            mask = dst < N_U
            d_local = dst[mask]
            s_local = src[mask] - N_U
            feats = v_bf
            dst_base = 0
            src_base = N_U

        n_dst = N_U
        cnt = np.bincount(d_local, minlength=n_dst)
        order = np.lexsort((np.arange(n_dst), cnt))
        rank = np.empty(n_dst, np.int64)
        rank[order] = np.arange(n_dst)

        spc = n_dst // N_CORES                      # 6250 slots per core
        r = np.arange(n_dst)
        cnt_mat = np.zeros((N_CORES, spc), np.int64)
        cnt_mat[r % N_CORES, r // N_CORES] = cnt[order]
        dst_mat = np.full((N_CORES, spc), -1, np.int64)
        dst_mat[r % N_CORES, r // N_CORES] = order + dst_base
        C = cnt_mat.max(axis=0)                     # canonical slot degrees

        for k in range(N_CORES):
            per_core_dsts[k].append(dst_mat[k])

        # ---- canonical windows + chunk packing (slots may straddle chunks)
        n_win = (spc + WIN - 1) // WIN
        pos_base = np.zeros(spc, np.int64)          # window-local row of slot's 1st edge
        win_ms0 = np.zeros(n_win, np.int64)         # global ms col base per window
        win_nb = np.zeros(n_win, np.int64)
        # per phase-local chunk: window-local first slot / window-local s col base
        chunks_col0 = []
        chunks_wscol0 = []
        chunks_win = []
        for w in range(n_win):
            s0, s1 = w * WIN, min((w + 1) * WIN, spc)
            Cw = C[s0:s1]
            cum = np.concatenate([[0], np.cumsum(Cw)])
            rows_win = int(cum[-1])
            nb = (rows_win + P - 1) // P
            pos_base[s0:s1] = cum[:-1]
            win_ms0[w] = ms_base
            win_nb[w] = nb
            chunks = []
            wsc = 0
            for b in range(nb):
                r0, r1 = b * P, min((b + 1) * P, rows_win)
                first = int(np.searchsorted(cum, r0, side="right")) - 1
                last = int(np.searchsorted(cum, r1, side="left")) - 1
                cols = last - first + 1
                chunks.append({"col0": first, "cols": cols, "wscol0": wsc})
                chunks_col0.append(first)
                chunks_wscol0.append(wsc)
                chunks_win.append(w)
                wsc += cols
            wlist.append({
                "phase": phase,
                "ns": s1 - s0,
                "nb": nb,
                "chunks": chunks,
                "ms0": ms_base,
                "msw": nb * D + wsc,
                "scw": wsc,
                "slot0": slot_base + s0,
            })
            ms_base += nb * D + wsc

        # ---- per-core edge placement (vectorized)
        grp = d_local
        sort_i = np.argsort(grp, kind="stable")
        grp_s = grp[sort_i]
        starts = np.r_[0, np.nonzero(np.diff(grp_s))[0] + 1]
        group_id = np.cumsum(np.r_[0, (np.diff(grp_s) != 0).astype(np.int64)])
        within = np.arange(len(grp_s)) - starts[group_id]
        e_rank = np.empty(len(grp), np.int64)
        e_rank[sort_i] = within

        win_chunk0 = np.r_[0, np.cumsum(win_nb)][:-1]
        e_core = rank[d_local] % N_CORES
        e_slot = rank[d_local] // N_CORES
        e_win = e_slot // WIN
        e_lpos = pos_base[e_slot] + e_rank
        e_chunk = win_chunk0[e_win] + e_lpos // P   # phase-local chunk id
        e_row = e_lpos % P
        cc0 = np.asarray(chunks_col0, np.int64)
        cw0 = np.asarray(chunks_wscol0, np.int64)
        cwin = np.asarray(chunks_win, np.int64)
        slot_local = e_slot - e_win * WIN
        # global ms col of the edge's feature block / of its S entry
        e_mcol = win_ms0[cwin[e_chunk]] + (e_chunk - win_chunk0[cwin[e_chunk]]) * D
        e_scol = (win_ms0[cwin[e_chunk]] + win_nb[cwin[e_chunk]] * D
                  + cw0[e_chunk] + slot_local - cc0[e_chunk])
        e_val = (cout[s_local + src_base] * cin[d_local + dst_base]).astype(np.float32)

        phase_w = ms_base - int(win_ms0[0])
        for k in range(N_CORES):
            m = e_core == k
            MS = np.zeros((P, phase_w), BF16)
            # feature rows: cols [e_mcol, e_mcol + D)
            fcol = (e_mcol[m] - int(win_ms0[0]))[:, None] + np.arange(D)[None, :]
            MS[e_row[m][:, None], fcol] = feats[s_local[m]]
            MS[e_row[m], e_scol[m] - int(win_ms0[0])] = e_val[m].astype(BF16)
            per_core_ms[k].append(MS)

        slot_base += spc

    totals = {
        "tot_ms": ms_base,
        "tot_slots": slot_base,
        "msw_max": max(w["msw"] for w in wlist),
    }

    per_core = []
    for k in range(N_CORES):
        ms_arr = np.concatenate(per_core_ms[k], axis=1)
        per_core.append({"ms": ms_arr, "dsts": per_core_dsts[k]})
        per_core_ms[k] = None
    return wlist, totals, per_core


# ------------------------------------------------------------------ device code
def _build_nc(wlist, totals):
    import concourse.bacc as bacc
    import concourse.bass as bass
    import concourse.mybir as mybir
    from concourse._compat import get_trn_type

    nc = bacc.Bacc(get_trn_type() or "TRN2", target_bir_lowering=False, debug=False)
    f32 = mybir.dt.float32
    bf16 = mybir.dt.bfloat16

    TOTMS = totals["tot_ms"]
    TS = totals["tot_slots"]
    MSWMAX = totals["msw_max"]

    ms_in = nc.dram_tensor("ms", [P, TOTMS], bf16, kind="ExternalInput")
    u_w = nc.dram_tensor("u_w", [D, D], f32, kind="ExternalInput")
    v_w = nc.dram_tensor("v_w", [D, D], f32, kind="ExternalInput")
    out = nc.dram_tensor("out", [P, TS], f32, kind="ExternalOutput")

    ms_sb = [nc.alloc_sbuf_tensor(f"ms{i}", [P, MSWMAX], bf16) for i in range(NBUF)]
    agg_sb = [nc.alloc_sbuf_tensor(f"agg{i}", [P, WIN], f32) for i in (0, 1)]
    stage_sb = nc.alloc_sbuf_tensor("stage", [P, TS], f32)
    w_sb = nc.alloc_sbuf_tensor("w", [P, 2, D], f32)

    agg_ps = [nc.alloc_psum_tensor(f"aps{i}", [P, WIN], f32) for i in (0, 1)]
    proj_ps = [nc.alloc_psum_tensor(f"pps{i}", [P, WIN], f32) for i in (0, 1)]

    sem_ld = nc.alloc_semaphore("ld")
    sem_s = [nc.alloc_semaphore(f"ssem{i}") for i in range(NBUF)]
    sem_mm = [nc.alloc_semaphore(f"mmsem{i}") for i in range(NBUF)]
    sem_agg = [nc.alloc_semaphore(f"aggsem{i}") for i in (0, 1)]
    sem_proj = [nc.alloc_semaphore(f"projsem{i}") for i in (0, 1)]
    sem_stage = [nc.alloc_semaphore(f"stsem{i}") for i in (0, 1)]

    NW = len(wlist)
    # cumulative semaphore targets (mm by mod-NBUF class; rest by parity)
    mm_counts = {}
    agg_counts = {}
    agg_counts_prior = {}
    stage_counts = {}
    stage_counts_prior = {}
    mm_c = [0] * NBUF
    agg_c = [0, 0]
    st_c = [0, 0]
    for wi in range(NW):
        b3 = wi % NBUF
        b2 = wi % 2
        mm_c[b3] += 1
        mm_counts[wi] = mm_c[b3]
        agg_counts_prior[wi] = agg_c[b2]
        agg_c[b2] += 1
        agg_counts[wi] = agg_c[b2]
        stage_counts_prior[wi] = st_c[b2]
        st_c[b2] += 1
        stage_counts[wi] = st_c[b2]

    with nc.Block() as block:
        @block.sync
        def _(sy: bass.BassEngine):
            sy.dma_start(w_sb[:, 0, :], u_w[:]).then_inc(sem_ld, 16)
            sy.dma_start(w_sb[:, 1, :], v_w[:]).then_inc(sem_ld, 16)
            for wi, went in enumerate(wlist):
                b3 = wi % NBUF
                if wi >= NBUF:
                    sy.wait_ge(sem_mm[b3], mm_counts[wi - NBUF])
                sy.dma_start(
                    ms_sb[b3][:, :went["msw"]],
                    ms_in[:, went["ms0"]:went["ms0"] + went["msw"]],
                ).then_inc(sem_s[b3], 16)
            sy.wait_ge(sem_ld, 32)

        @block.tensor
        def _(te):
            te.wait_ge(sem_ld, 32)          # both weight matrices resident
            s_seen = [0] * NBUF
            for wi, went in enumerate(wlist):
                b3 = wi % NBUF
                b2 = wi % 2
                s_seen[b3] += 16
                te.wait_ge(sem_s[b3], s_seen[b3])
                if wi >= 2:
                    te.wait_ge(sem_agg[b2], agg_counts_prior[wi])
                nb = went["nb"]
                soff = nb * D
                for ci, ch in enumerate(went["chunks"]):
                    sc = soff + ch["wscol0"]
                    mm = te.matmul(
                        out=agg_ps[b2][:, ch["col0"]:ch["col0"] + ch["cols"]],
                        lhsT=ms_sb[b3][:, ci * D:(ci + 1) * D],
                        rhs=ms_sb[b3][:, sc:sc + ch["cols"]],
                        start=(ci == 0),
                        stop=(ci == nb - 1),
                    )
                    if ci == nb - 1:
                        mm.then_inc(sem_mm[b3], 1)
                te.wait_ge(sem_agg[b2], agg_counts[wi])
                if wi >= 2:
                    te.wait_ge(sem_stage[b2], stage_counts_prior[wi])
                ns = went["ns"]
                te.matmul(
                    out=proj_ps[b2][:, :ns],
                    lhsT=w_sb[:, went["phase"], :],
                    rhs=agg_sb[b2][:, :ns],
                    start=True, stop=True,
                ).then_inc(sem_proj[b2], 1)

        @block.vector
        def _(ve):
            mm_seen = [0] * NBUF
            pr_seen = [0, 0]
            for wi, went in enumerate(wlist):
                b3 = wi % NBUF
                b2 = wi % 2
                ns = went["ns"]
                mm_seen[b3] += 1
                ve.wait_ge(sem_mm[b3], mm_seen[b3])
                ve.tensor_copy(out=agg_sb[b2][:, :ns],
                               in_=agg_ps[b2][:, :ns]).then_inc(sem_agg[b2], 1)
                pr_seen[b2] += 1
                ve.wait_ge(sem_proj[b2], pr_seen[b2])
                ve.tensor_copy(
                    out=stage_sb[:, went["slot0"]:went["slot0"] + ns],
                    in_=proj_ps[b2][:, :ns],
                ).then_inc(sem_stage[b2], 1)

        @block.scalar
        def _(sc):
            st_seen = [0, 0]
            total = 0
            for wi, went in enumerate(wlist):
                b2 = wi % 2
                st_seen[b2] += 1
                sc.wait_ge(sem_stage[b2], st_seen[b2])
                ns = went["ns"]
                sc.dma_start(
                    out[:, went["slot0"]:went["slot0"] + ns],
                    stage_sb[:, went["slot0"]:went["slot0"] + ns],
                ).then_inc(sem_ld, 16)
                total += 16
            sc.wait_ge(sem_ld, 32 + total)

    nc.compile()
    return nc


# ---------------------------------------------------------------------- kernel
def kernel(u_f, v_f, u_w, v_w, src, dst):
    from concourse.bass_utils import run_bass_kernel_spmd

    src = np.asarray(src)
    dst = np.asarray(dst)
    u_bf = np.asarray(u_f, np.float32).astype(BF16)
    v_bf = np.asarray(v_f, np.float32).astype(BF16)

    deg_out = np.bincount(src, minlength=N).astype(np.float32)
    deg_in = np.bincount(dst, minlength=N).astype(np.float32)
    cout = np.maximum(deg_out, 1.0) ** -0.5
    cin = np.maximum(deg_in, 1.0) ** -0.5

    wlist, totals, per_core = _build_layout(src, dst, cout, cin, u_bf, v_bf)

    nc = _build_nc(wlist, totals)
    in_maps = []
    for k in range(N_CORES):
        in_maps.append({
            "ms": per_core[k]["ms"],
            "u_w": np.asarray(u_w, np.float32),
            "v_w": np.asarray(v_w, np.float32),
        })
    trace = bool(os.environ.get("KERNEL_TRACE"))
    res = run_bass_kernel_spmd(nc, in_maps, core_ids=list(range(N_CORES)),
                               trace=trace)
    if trace:
        print(f"HW exec time: {res.exec_time_ns} ns")
        kernel.last_profile = res.profile_json

    out_full = np.zeros((N, D), np.float32)
    for k in range(N_CORES):
        fm = res.results[k]["out"]            # [128, tot_slots]
        rows = np.ascontiguousarray(fm.T)     # [tot_slots, 128]
        slot0 = 0
        for phase in range(2):
            dsts = per_core[k]["dsts"][phase]
            nslots = len(dsts)
            valid = dsts >= 0
            out_full[dsts[valid]] = rows[slot0:slot0 + nslots][valid]
            slot0 += nslots
    return out_full


# revision 6
# speedup vs baseline: 1.0182x; 1.0182x over previous
"""Bipartite GCN message-passing kernel for 8 Trainium2 NeuronCores.

Math (reference): rst = deg_in^-1/2 * segsum_dst( (node_f @ W_side) * deg_out^-1/2 [src] )
Refactor (projection is linear, graph strictly bipartite):
    rst[d] = ( sum_{e->d} c_e * f_raw[src_e] ) @ W_side(d),
    c_e = deg_out[src]^-1/2 * deg_in[dst]^-1/2

Division of labor:
  HOST (layout / index math only — no feature arithmetic):
    degree counts, per-core dst dealing, canonical chunk schedule, and a
    bf16 edge-major re-layout of the raw feature rows (M tiles = f[src_e]
    placed at its schedule position) plus compact scatter blocks S holding
    c_e, merged into one stream per window.  This replaces the v1
    device-side dma_gather, whose GPSIMD descriptor generation (~8 ns/edge,
    serial on the Q7s) was a hard 1.6 ms floor.
  DEVICE (all feature FLOPs):
    per window: stream the merged M|S tile sequentially at DMA line rate,
    aggregate PSUM[feat, dst_slot] += M_chunk[128e,128f].T @ S_chunk[128e,cols]
    (bf16 matmuls, fp32 accumulate), then project with W_side (fp32) and
    stream out the [128, slots] feature-major result window by window.

Sharding: dst nodes dealt round-robin (degree-sorted) to 8 cores ->
identical compile-time schedule per core (SPMD), no collectives.
"""
import sys
import os

for _p in ("/opt/trn_rl_repo",):
    if _p not in sys.path and os.path.isdir(_p):
        sys.path.insert(0, _p)

import numpy as np
import ml_dtypes

BF16 = ml_dtypes.bfloat16

N_U = 50000
N_V = 50000
N = N_U + N_V
D = 128
E = 1600000
N_CORES = 8
WIN = 512             # dst slots per PSUM window
P = 128
NBUF = 3              # input stream buffers


# ----------------------------------------------------------------- host layout
def _build_layout(src, dst, cout, cin, u_bf, v_bf):
    """Canonical schedule + per-core merged M|S stream data.

    Returns (wlist, totals, per_core). wlist is the compile-time window
    list (identical across cores); per_core holds the merged ms array +
    the slot -> global dst id mapping.
    """
    wlist = []
    per_core_ms = [[] for _ in range(N_CORES)]
    per_core_dsts = [[] for _ in range(N_CORES)]

    ms_base = 0           # global ms column counter
    slot_base = 0         # global output slot counter

    for phase in range(2):
        if phase == 0:    # dsts are v-nodes, sources u-side
            mask = dst >= N_U
            d_local = dst[mask] - N_U
            s_local = src[mask]
            feats = u_bf
            dst_base = N_U
            src_base = 0
        else:             # dsts are u-nodes, sources v-side
            mask = dst < N_U
            d_local = dst[mask]
            s_local = src[mask] - N_U
            feats = v_bf
            dst_base = 0
            src_base = N_U

        n_dst = N_U
        cnt = np.bincount(d_local, minlength=n_dst)
        order = np.lexsort((np.arange(n_dst), cnt))
        rank = np.empty(n_dst, np.int64)
        rank[order] = np.arange(n_dst)

        spc = n_dst // N_CORES                      # 6250 slots per core
        r = np.arange(n_dst)
        cnt_mat = np.zeros((N_CORES, spc), np.int64)
        cnt_mat[r % N_CORES, r // N_CORES] = cnt[order]
        dst_mat = np.full((N_CORES, spc), -1, np.int64)
        dst_mat[r % N_CORES, r // N_CORES] = order + dst_base
        C = cnt_mat.max(axis=0)                     # canonical slot degrees

        for k in range(N_CORES):
            per_core_dsts[k].append(dst_mat[k])

        # ---- canonical windows + chunk packing (slots may straddle chunks)
        n_win = (spc + WIN - 1) // WIN
        pos_base = np.zeros(spc, np.int64)          # window-local row of slot's 1st edge
        win_ms0 = np.zeros(n_win, np.int64)         # global ms col base per window
        win_nb = np.zeros(n_win, np.int64)
        # per phase-local chunk: window-local first slot / window-local s col base
        chunks_col0 = []
        chunks_wscol0 = []
        chunks_win = []
        for w in range(n_win):
            s0, s1 = w * WIN, min((w + 1) * WIN, spc)
            Cw = C[s0:s1]
            cum = np.concatenate([[0], np.cumsum(Cw)])
            rows_win = int(cum[-1])
            nb = (rows_win + P - 1) // P
            pos_base[s0:s1] = cum[:-1]
            win_ms0[w] = ms_base
            win_nb[w] = nb
            chunks = []
            wsc = 0
            for b in range(nb):
                r0, r1 = b * P, min((b + 1) * P, rows_win)
                first = int(np.searchsorted(cum, r0, side="right")) - 1
                last = int(np.searchsorted(cum, r1, side="left")) - 1
                cols = last - first + 1
                chunks.append({"col0": first, "cols": cols, "wscol0": wsc})
                chunks_col0.append(first)
                chunks_wscol0.append(wsc)
                chunks_win.append(w)
                wsc += cols
            wlist.append({
                "phase": phase,
                "ns": s1 - s0,
                "nb": nb,
                "chunks": chunks,
                "ms0": ms_base,
                "msw": nb * D + wsc,
                "scw": wsc,
                "slot0": slot_base + s0,
            })
            ms_base += nb * D + wsc

        # ---- per-core edge placement (vectorized)
        grp = d_local
        sort_i = np.argsort(grp, kind="stable")
        grp_s = grp[sort_i]
        starts = np.r_[0, np.nonzero(np.diff(grp_s))[0] + 1]
        group_id = np.cumsum(np.r_[0, (np.diff(grp_s) != 0).astype(np.int64)])
        within = np.arange(len(grp_s)) - starts[group_id]
        e_rank = np.empty(len(grp), np.int64)
        e_rank[sort_i] = within

        win_chunk0 = np.r_[0, np.cumsum(win_nb)][:-1]
        e_core = rank[d_local] % N_CORES
        e_slot = rank[d_local] // N_CORES
        e_win = e_slot // WIN
        e_lpos = pos_base[e_slot] + e_rank
        e_chunk = win_chunk0[e_win] + e_lpos // P   # phase-local chunk id
        e_row = e_lpos % P
        cc0 = np.asarray(chunks_col0, np.int64)
        cw0 = np.asarray(chunks_wscol0, np.int64)
        cwin = np.asarray(chunks_win, np.int64)
        slot_local = e_slot - e_win * WIN
        # global ms col of the edge's feature block / of its S entry
        e_mcol = win_ms0[cwin[e_chunk]] + (e_chunk - win_chunk0[cwin[e_chunk]]) * D
        e_scol = (win_ms0[cwin[e_chunk]] + win_nb[cwin[e_chunk]] * D
                  + cw0[e_chunk] + slot_local - cc0[e_chunk])
        e_val = (cout[s_local + src_base] * cin[d_local + dst_base]).astype(np.float32)

        phase_w = ms_base - int(win_ms0[0])
        for k in range(N_CORES):
            m = e_core == k
            MS = np.zeros((P, phase_w), BF16)
            # feature rows: cols [e_mcol, e_mcol + D)
            fcol = (e_mcol[m] - int(win_ms0[0]))[:, None] + np.arange(D)[None, :]
            MS[e_row[m][:, None], fcol] = feats[s_local[m]]
            MS[e_row[m], e_scol[m] - int(win_ms0[0])] = e_val[m].astype(BF16)
            per_core_ms[k].append(MS)

        slot_base += spc

    totals = {
        "tot_ms": ms_base,
        "tot_slots": slot_base,
        "msw_max": max(w["msw"] for w in wlist),
    }

    per_core = []
    for k in range(N_CORES):
        ms_arr = np.concatenate(per_core_ms[k], axis=1)
        per_core.append({"ms": ms_arr, "dsts": per_core_dsts[k]})
        per_core_ms[k] = None
    return wlist, totals, per_core


# ------------------------------------------------------------------ device code
def _build_nc(wlist, totals):
    import concourse.bacc as bacc
    import concourse.bass as bass
    import concourse.mybir as mybir
    from concourse._compat import get_trn_type

    nc = bacc.Bacc(get_trn_type() or "TRN2", target_bir_lowering=False, debug=False)
    f32 = mybir.dt.float32
    bf16 = mybir.dt.bfloat16

    TOTMS = totals["tot_ms"]
    TS = totals["tot_slots"]
    MSWMAX = totals["msw_max"]

    ms_in = nc.dram_tensor("ms", [P, TOTMS], bf16, kind="ExternalInput")
    u_w = nc.dram_tensor("u_w", [D, D], f32, kind="ExternalInput")
    v_w = nc.dram_tensor("v_w", [D, D], f32, kind="ExternalInput")
    out = nc.dram_tensor("out", [P, TS], bf16, kind="ExternalOutput")

    ms_sb = [nc.alloc_sbuf_tensor(f"ms{i}", [P, MSWMAX], bf16) for i in range(NBUF)]
    agg_sb = [nc.alloc_sbuf_tensor(f"agg{i}", [P, WIN], f32) for i in (0, 1)]
    stage_sb = nc.alloc_sbuf_tensor("stage", [P, TS], bf16)
    w_sb = nc.alloc_sbuf_tensor("w", [P, 2, D], f32)

    agg_ps = [nc.alloc_psum_tensor(f"aps{i}", [P, WIN], f32) for i in (0, 1)]
    proj_ps = [nc.alloc_psum_tensor(f"pps{i}", [P, WIN], f32) for i in (0, 1)]

    sem_ld = nc.alloc_semaphore("ld")
    sem_s = [nc.alloc_semaphore(f"ssem{i}") for i in range(NBUF)]
    sem_mm = [nc.alloc_semaphore(f"mmsem{i}") for i in range(NBUF)]
    sem_agg = [nc.alloc_semaphore(f"aggsem{i}") for i in (0, 1)]
    sem_proj = [nc.alloc_semaphore(f"projsem{i}") for i in (0, 1)]
    sem_stage = [nc.alloc_semaphore(f"stsem{i}") for i in (0, 1)]

    NW = len(wlist)
    # cumulative semaphore targets (mm by mod-NBUF class; rest by parity)
    mm_counts = {}
    agg_counts = {}
    agg_counts_prior = {}
    stage_counts = {}
    stage_counts_prior = {}
    mm_c = [0] * NBUF
    agg_c = [0, 0]
    st_c = [0, 0]
    for wi in range(NW):
        b3 = wi % NBUF
        b2 = wi % 2
        mm_c[b3] += 1
        mm_counts[wi] = mm_c[b3]
        agg_counts_prior[wi] = agg_c[b2]
        agg_c[b2] += 1
        agg_counts[wi] = agg_c[b2]
        stage_counts_prior[wi] = st_c[b2]
        st_c[b2] += 1
        stage_counts[wi] = st_c[b2]

    with nc.Block() as block:
        @block.sync
        def _(sy: bass.BassEngine):
            sy.dma_start(w_sb[:, 0, :], u_w[:]).then_inc(sem_ld, 16)
            sy.dma_start(w_sb[:, 1, :], v_w[:]).then_inc(sem_ld, 16)
            for wi, went in enumerate(wlist):
                b3 = wi % NBUF
                if wi >= NBUF:
                    sy.wait_ge(sem_mm[b3], mm_counts[wi - NBUF])
                sy.dma_start(
                    ms_sb[b3][:, :went["msw"]],
                    ms_in[:, went["ms0"]:went["ms0"] + went["msw"]],
                ).then_inc(sem_s[b3], 16)
            sy.wait_ge(sem_ld, 32)

        @block.tensor
        def _(te):
            te.wait_ge(sem_ld, 32)          # both weight matrices resident
            s_seen = [0] * NBUF
            for wi, went in enumerate(wlist):
                b3 = wi % NBUF
                b2 = wi % 2
                s_seen[b3] += 16
                te.wait_ge(sem_s[b3], s_seen[b3])
                if wi >= 2:
                    te.wait_ge(sem_agg[b2], agg_counts_prior[wi])
                nb = went["nb"]
                soff = nb * D
                for ci, ch in enumerate(went["chunks"]):
                    sc = soff + ch["wscol0"]
                    mm = te.matmul(
                        out=agg_ps[b2][:, ch["col0"]:ch["col0"] + ch["cols"]],
                        lhsT=ms_sb[b3][:, ci * D:(ci + 1) * D],
                        rhs=ms_sb[b3][:, sc:sc + ch["cols"]],
                        start=(ci == 0),
                        stop=(ci == nb - 1),
                    )
                    if ci == nb - 1:
                        mm.then_inc(sem_mm[b3], 1)
                te.wait_ge(sem_agg[b2], agg_counts[wi])
                if wi >= 2:
                    te.wait_ge(sem_stage[b2], stage_counts_prior[wi])
                ns = went["ns"]
                te.matmul(
                    out=proj_ps[b2][:, :ns],
                    lhsT=w_sb[:, went["phase"], :],
                    rhs=agg_sb[b2][:, :ns],
                    start=True, stop=True,
                ).then_inc(sem_proj[b2], 1)

        @block.vector
        def _(ve):
            mm_seen = [0] * NBUF
            for wi, went in enumerate(wlist):
                b3 = wi % NBUF
                b2 = wi % 2
                ns = went["ns"]
                mm_seen[b3] += 1
                ve.wait_ge(sem_mm[b3], mm_seen[b3])
                ve.tensor_copy(out=agg_sb[b2][:, :ns],
                               in_=agg_ps[b2][:, :ns]).then_inc(sem_agg[b2], 1)

        @block.scalar
        def _(sc):
            pr_seen = [0, 0]
            total = 0
            # group window outputs into ~4 large DMAs
            n_groups = 4
            bounds = [round(g * NW / n_groups) for g in range(1, n_groups + 1)]
            gstart = 0
            for wi, went in enumerate(wlist):
                b2 = wi % 2
                ns = went["ns"]
                pr_seen[b2] += 1
                sc.wait_ge(sem_proj[b2], pr_seen[b2])
                sc.copy(
                    out=stage_sb[:, went["slot0"]:went["slot0"] + ns],
                    in_=proj_ps[b2][:, :ns],
                ).then_inc(sem_stage[b2], 1)
                if wi + 1 in bounds:
                    c0 = wlist[gstart]["slot0"]
                    c1 = went["slot0"] + ns
                    sc.dma_start(
                        out[:, c0:c1], stage_sb[:, c0:c1]
                    ).then_inc(sem_ld, 16)
                    total += 16
                    gstart = wi + 1
            sc.wait_ge(sem_ld, 32 + total)

    nc.compile()
    return nc


# ---------------------------------------------------------------------- kernel
def kernel(u_f, v_f, u_w, v_w, src, dst):
    from concourse.bass_utils import run_bass_kernel_spmd

    src = np.asarray(src)
    dst = np.asarray(dst)
    u_bf = np.asarray(u_f, np.float32).astype(BF16)
    v_bf = np.asarray(v_f, np.float32).astype(BF16)

    deg_out = np.bincount(src, minlength=N).astype(np.float32)
    deg_in = np.bincount(dst, minlength=N).astype(np.float32)
    cout = np.maximum(deg_out, 1.0) ** -0.5
    cin = np.maximum(deg_in, 1.0) ** -0.5

    wlist, totals, per_core = _build_layout(src, dst, cout, cin, u_bf, v_bf)

    nc = _build_nc(wlist, totals)
    in_maps = []
    for k in range(N_CORES):
        in_maps.append({
            "ms": per_core[k]["ms"],
            "u_w": np.asarray(u_w, np.float32),
            "v_w": np.asarray(v_w, np.float32),
        })
    trace = bool(os.environ.get("KERNEL_TRACE"))
    res = run_bass_kernel_spmd(nc, in_maps, core_ids=list(range(N_CORES)),
                               trace=trace)
    if trace:
        print(f"HW exec time: {res.exec_time_ns} ns")
        kernel.last_profile = res.profile_json

    out_full = np.zeros((N, D), np.float32)
    for k in range(N_CORES):
        fm = res.results[k]["out"]            # [128, tot_slots] bf16
        rows = np.ascontiguousarray(fm.T).astype(np.float32)   # [tot_slots, 128]
        slot0 = 0
        for phase in range(2):
            dsts = per_core[k]["dsts"][phase]
            nslots = len(dsts)
            valid = dsts >= 0
            out_full[dsts[valid]] = rows[slot0:slot0 + nslots][valid]
            slot0 += nslots
    return out_full


# revision 9
# speedup vs baseline: 1.1364x; 1.1161x over previous
"""Bipartite GCN message-passing kernel for 8 Trainium2 NeuronCores.

Math (reference): rst = deg_in^-1/2 * segsum_dst( (node_f @ W_side) * deg_out^-1/2 [src] )
Refactor (projection is linear, graph strictly bipartite):
    rst[d] = ( sum_{e->d} c_e * f_raw[src_e] ) @ W_side(d),
    c_e = deg_out[src]^-1/2 * deg_in[dst]^-1/2

Division of labor:
  HOST (layout / index math only — no feature arithmetic):
    degree counts, per-core dst dealing, canonical chunk schedule, and a
    bf16 edge-major re-layout of the raw feature rows (M tiles = f[src_e]
    placed at its schedule position) plus compact scatter blocks S holding
    c_e, merged into one stream per window.  This replaces the v1
    device-side dma_gather, whose GPSIMD descriptor generation (~8 ns/edge,
    serial on the Q7s) was a hard 1.6 ms floor.
  DEVICE (all feature FLOPs):
    per window: stream the merged M|S tile sequentially at DMA line rate,
    aggregate PSUM[feat, dst_slot] += M_chunk[128e,128f].T @ S_chunk[128e,cols]
    (bf16 matmuls, fp32 accumulate), then project with W_side (fp32) and
    stream out the [128, slots] feature-major result window by window.

Sharding: dst nodes dealt round-robin (degree-sorted) to 8 cores ->
identical compile-time schedule per core (SPMD), no collectives.
"""
import sys
import os

for _p in ("/opt/trn_rl_repo",):
    if _p not in sys.path and os.path.isdir(_p):
        sys.path.insert(0, _p)

import numpy as np
import ml_dtypes

BF16 = ml_dtypes.bfloat16

N_U = 50000
N_V = 50000
N = N_U + N_V
D = 128
E = 1600000
N_CORES = 8
WIN = 512             # dst slots per PSUM window
P = 128
NBUF = 4              # input stream buffers


# ----------------------------------------------------------------- host layout
def _build_layout(src, dst, cout, cin, u_bf, v_bf):
    """Canonical schedule + per-core merged M|S stream data.

    Returns (wlist, totals, per_core). wlist is the compile-time window
    list (identical across cores); per_core holds the merged ms array +
    the slot -> global dst id mapping.
    """
    wlist = []
    per_core_ms = [[] for _ in range(N_CORES)]
    per_core_dsts = [[] for _ in range(N_CORES)]

    ms_base = 0           # global ms column counter
    slot_base = 0         # global output slot counter

    for phase in range(2):
        if phase == 0:    # dsts are v-nodes, sources u-side
            mask = dst >= N_U
            d_local = dst[mask] - N_U
            s_local = src[mask]
            feats = u_bf
            dst_base = N_U
            src_base = 0
        else:             # dsts are u-nodes, sources v-side
            mask = dst < N_U
            d_local = dst[mask]
            s_local = src[mask] - N_U
            feats = v_bf
            dst_base = 0
            src_base = N_U

        n_dst = N_U
        cnt = np.bincount(d_local, minlength=n_dst)
        order = np.lexsort((np.arange(n_dst), cnt))
        rank = np.empty(n_dst, np.int64)
        rank[order] = np.arange(n_dst)

        spc = n_dst // N_CORES                      # 6250 slots per core
        r = np.arange(n_dst)
        cnt_mat = np.zeros((N_CORES, spc), np.int64)
        cnt_mat[r % N_CORES, r // N_CORES] = cnt[order]
        dst_mat = np.full((N_CORES, spc), -1, np.int64)
        dst_mat[r % N_CORES, r // N_CORES] = order + dst_base
        C = cnt_mat.max(axis=0)                     # canonical slot degrees

        for k in range(N_CORES):
            per_core_dsts[k].append(dst_mat[k])

        # ---- canonical windows + chunk packing (slots may straddle chunks)
        n_win = (spc + WIN - 1) // WIN
        pos_base = np.zeros(spc, np.int64)          # window-local row of slot's 1st edge
        win_ms0 = np.zeros(n_win, np.int64)         # global ms col base per window
        win_nb = np.zeros(n_win, np.int64)
        # per phase-local chunk: window-local first slot / window-local s col base
        chunks_col0 = []
        chunks_wscol0 = []
        chunks_win = []
        for w in range(n_win):
            s0, s1 = w * WIN, min((w + 1) * WIN, spc)
            Cw = C[s0:s1]
            cum = np.concatenate([[0], np.cumsum(Cw)])
            rows_win = int(cum[-1])
            nb = (rows_win + P - 1) // P
            pos_base[s0:s1] = cum[:-1]
            win_ms0[w] = ms_base
            win_nb[w] = nb
            chunks = []
            wsc = 0
            for b in range(nb):
                r0, r1 = b * P, min((b + 1) * P, rows_win)
                first = int(np.searchsorted(cum, r0, side="right")) - 1
                last = int(np.searchsorted(cum, r1, side="left")) - 1
                cols = last - first + 1
                chunks.append({"col0": first, "cols": cols, "wscol0": wsc})
                chunks_col0.append(first)
                chunks_wscol0.append(wsc)
                chunks_win.append(w)
                wsc += cols
            wlist.append({
                "phase": phase,
                "ns": s1 - s0,
                "nb": nb,
                "chunks": chunks,
                "ms0": ms_base,
                "msw": nb * D + wsc,
                "scw": wsc,
                "slot0": slot_base + s0,
            })
            ms_base += nb * D + wsc

        # ---- per-core edge placement (vectorized)
        grp = d_local
        sort_i = np.argsort(grp, kind="stable")
        grp_s = grp[sort_i]
        starts = np.r_[0, np.nonzero(np.diff(grp_s))[0] + 1]
        group_id = np.cumsum(np.r_[0, (np.diff(grp_s) != 0).astype(np.int64)])
        within = np.arange(len(grp_s)) - starts[group_id]
        e_rank = np.empty(len(grp), np.int64)
        e_rank[sort_i] = within

        win_chunk0 = np.r_[0, np.cumsum(win_nb)][:-1]
        e_core = rank[d_local] % N_CORES
        e_slot = rank[d_local] // N_CORES
        e_win = e_slot // WIN
        e_lpos = pos_base[e_slot] + e_rank
        e_chunk = win_chunk0[e_win] + e_lpos // P   # phase-local chunk id
        e_row = e_lpos % P
        cc0 = np.asarray(chunks_col0, np.int64)
        cw0 = np.asarray(chunks_wscol0, np.int64)
        cwin = np.asarray(chunks_win, np.int64)
        slot_local = e_slot - e_win * WIN
        # global ms col of the edge's feature block / of its S entry
        e_mcol = win_ms0[cwin[e_chunk]] + (e_chunk - win_chunk0[cwin[e_chunk]]) * D
        e_scol = (win_ms0[cwin[e_chunk]] + win_nb[cwin[e_chunk]] * D
                  + cw0[e_chunk] + slot_local - cc0[e_chunk])
        e_val = (cout[s_local + src_base] * cin[d_local + dst_base]).astype(np.float32)

        phase_w = ms_base - int(win_ms0[0])
        for k in range(N_CORES):
            m = e_core == k
            MS = np.zeros((P, phase_w), BF16)
            # feature rows: cols [e_mcol, e_mcol + D)
            fcol = (e_mcol[m] - int(win_ms0[0]))[:, None] + np.arange(D)[None, :]
            MS[e_row[m][:, None], fcol] = feats[s_local[m]]
            MS[e_row[m], e_scol[m] - int(win_ms0[0])] = e_val[m].astype(BF16)
            per_core_ms[k].append(MS)

        slot_base += spc

    totals = {
        "tot_ms": ms_base,
        "tot_slots": slot_base,
        "msw_max": max(w["msw"] for w in wlist),
    }

    per_core = []
    for k in range(N_CORES):
        ms_arr = np.concatenate(per_core_ms[k], axis=1)
        per_core.append({"ms": ms_arr, "dsts": per_core_dsts[k]})
        per_core_ms[k] = None
    return wlist, totals, per_core


# ------------------------------------------------------------------ device code
def _build_nc(wlist, totals):
    import concourse.bacc as bacc
    import concourse.bass as bass
    import concourse.mybir as mybir
    from concourse._compat import get_trn_type

    nc = bacc.Bacc(get_trn_type() or "TRN2", target_bir_lowering=False, debug=False)
    f32 = mybir.dt.float32
    bf16 = mybir.dt.bfloat16

    TOTMS = totals["tot_ms"]
    TS = totals["tot_slots"]
    MSWMAX = totals["msw_max"]

    ms_in = nc.dram_tensor("ms", [P, TOTMS], bf16, kind="ExternalInput")
    u_w = nc.dram_tensor("u_w", [D, D], f32, kind="ExternalInput")
    v_w = nc.dram_tensor("v_w", [D, D], f32, kind="ExternalInput")
    out = nc.dram_tensor("out", [P, TS], bf16, kind="ExternalOutput")

    ms_sb = [nc.alloc_sbuf_tensor(f"ms{i}", [P, MSWMAX], bf16) for i in range(NBUF)]
    agg_sb = [nc.alloc_sbuf_tensor(f"agg{i}", [P, WIN], f32) for i in (0, 1)]
    stage_sb = nc.alloc_sbuf_tensor("stage", [P, TS], bf16)
    w_sb = nc.alloc_sbuf_tensor("w", [P, 2, D], f32)

    agg_ps = [nc.alloc_psum_tensor(f"aps{i}", [P, WIN], f32) for i in (0, 1)]
    proj_ps = [nc.alloc_psum_tensor(f"pps{i}", [P, WIN], f32) for i in (0, 1)]

    sem_ld = nc.alloc_semaphore("ld")
    sem_s = [nc.alloc_semaphore(f"ssem{i}") for i in range(NBUF)]
    sem_mm = [nc.alloc_semaphore(f"mmsem{i}") for i in range(NBUF)]
    sem_agg = [nc.alloc_semaphore(f"aggsem{i}") for i in (0, 1)]
    sem_proj = [nc.alloc_semaphore(f"projsem{i}") for i in (0, 1)]
    sem_stage = [nc.alloc_semaphore(f"stsem{i}") for i in (0, 1)]

    NW = len(wlist)
    # cumulative semaphore targets (mm by mod-NBUF class; rest by parity)
    mm_counts = {}
    agg_counts = {}
    agg_counts_prior = {}
    stage_counts = {}
    stage_counts_prior = {}
    mm_c = [0] * NBUF
    agg_c = [0, 0]
    st_c = [0, 0]
    for wi in range(NW):
        b3 = wi % NBUF
        b2 = wi % 2
        mm_c[b3] += 1
        mm_counts[wi] = mm_c[b3]
        agg_counts_prior[wi] = agg_c[b2]
        agg_c[b2] += 1
        agg_counts[wi] = agg_c[b2]
        stage_counts_prior[wi] = st_c[b2]
        st_c[b2] += 1
        stage_counts[wi] = st_c[b2]

    with nc.Block() as block:
        @block.sync
        def _(sy: bass.BassEngine):
            sy.dma_start(w_sb[:, 0, :], u_w[:]).then_inc(sem_ld, 16)
            sy.dma_start(w_sb[:, 1, :], v_w[:]).then_inc(sem_ld, 16)
            for wi, went in enumerate(wlist):
                b3 = wi % NBUF
                if wi >= NBUF:
                    sy.wait_ge(sem_mm[b3], mm_counts[wi - NBUF])
                sy.dma_start(
                    ms_sb[b3][:, :went["msw"]],
                    ms_in[:, went["ms0"]:went["ms0"] + went["msw"]],
                ).then_inc(sem_s[b3], 16)
            sy.wait_ge(sem_ld, 32)

        @block.tensor
        def _(te):
            te.wait_ge(sem_ld, 32)          # both weight matrices resident
            s_seen = [0] * NBUF

            def proj(j):
                # deferred one window so the vector agg copy overlaps chunks
                wj = wlist[j]
                p2 = j % 2
                te.wait_ge(sem_agg[p2], agg_counts[j])
                if j >= 2:
                    te.wait_ge(sem_stage[p2], stage_counts_prior[j])
                te.matmul(
                    out=proj_ps[p2][:, :wj["ns"]],
                    lhsT=w_sb[:, wj["phase"], :],
                    rhs=agg_sb[p2][:, :wj["ns"]],
                    start=True, stop=True,
                ).then_inc(sem_proj[p2], 1)

            for wi, went in enumerate(wlist):
                b3 = wi % NBUF
                b2 = wi % 2
                s_seen[b3] += 16
                te.wait_ge(sem_s[b3], s_seen[b3])
                if wi >= 2:
                    te.wait_ge(sem_agg[b2], agg_counts_prior[wi])
                nb = went["nb"]
                soff = nb * D
                for ci, ch in enumerate(went["chunks"]):
                    sc = soff + ch["wscol0"]
                    mm = te.matmul(
                        out=agg_ps[b2][:, ch["col0"]:ch["col0"] + ch["cols"]],
                        lhsT=ms_sb[b3][:, ci * D:(ci + 1) * D],
                        rhs=ms_sb[b3][:, sc:sc + ch["cols"]],
                        start=(ci == 0),
                        stop=(ci == nb - 1),
                    )
                    if ci == nb - 1:
                        mm.then_inc(sem_mm[b3], 1)
                if wi >= 1:
                    proj(wi - 1)
            proj(len(wlist) - 1)

        @block.vector
        def _(ve):
            mm_seen = [0] * NBUF
            for wi, went in enumerate(wlist):
                b3 = wi % NBUF
                b2 = wi % 2
                ns = went["ns"]
                mm_seen[b3] += 1
                ve.wait_ge(sem_mm[b3], mm_seen[b3])
                ve.tensor_copy(out=agg_sb[b2][:, :ns],
                               in_=agg_ps[b2][:, :ns]).then_inc(sem_agg[b2], 1)

        @block.scalar
        def _(sc):
            pr_seen = [0, 0]
            total = 0
            # large output DMA groups, with a tiny final group to trim the tail
            bounds = [9, 15, 20, 24, NW]
            gstart = 0
            for wi, went in enumerate(wlist):
                b2 = wi % 2
                ns = went["ns"]
                pr_seen[b2] += 1
                sc.wait_ge(sem_proj[b2], pr_seen[b2])
                sc.copy(
                    out=stage_sb[:, went["slot0"]:went["slot0"] + ns],
                    in_=proj_ps[b2][:, :ns],
                ).then_inc(sem_stage[b2], 1)
                if wi + 1 in bounds:
                    c0 = wlist[gstart]["slot0"]
                    c1 = went["slot0"] + ns
                    sc.dma_start(
                        out[:, c0:c1], stage_sb[:, c0:c1]
                    ).then_inc(sem_ld, 16)
                    total += 16
                    gstart = wi + 1
            sc.wait_ge(sem_ld, 32 + total)

    nc.compile()
    return nc


# ---------------------------------------------------------------------- kernel
def kernel(u_f, v_f, u_w, v_w, src, dst):
    from concourse.bass_utils import run_bass_kernel_spmd

    src = np.asarray(src)
    dst = np.asarray(dst)
    u_bf = np.asarray(u_f, np.float32).astype(BF16)
    v_bf = np.asarray(v_f, np.float32).astype(BF16)

    deg_out = np.bincount(src, minlength=N).astype(np.float32)
    deg_in = np.bincount(dst, minlength=N).astype(np.float32)
    cout = np.maximum(deg_out, 1.0) ** -0.5
    cin = np.maximum(deg_in, 1.0) ** -0.5

    wlist, totals, per_core = _build_layout(src, dst, cout, cin, u_bf, v_bf)

    nc = _build_nc(wlist, totals)
    in_maps = []
    for k in range(N_CORES):
        in_maps.append({
            "ms": per_core[k]["ms"],
            "u_w": np.asarray(u_w, np.float32),
            "v_w": np.asarray(v_w, np.float32),
        })
    trace = bool(os.environ.get("KERNEL_TRACE"))
    res = run_bass_kernel_spmd(nc, in_maps, core_ids=list(range(N_CORES)),
                               trace=trace)
    if trace:
        print(f"HW exec time: {res.exec_time_ns} ns")
        kernel.last_profile = res.profile_json

    out_full = np.zeros((N, D), np.float32)
    for k in range(N_CORES):
        fm = res.results[k]["out"]            # [128, tot_slots] bf16
        rows = np.ascontiguousarray(fm.T).astype(np.float32)   # [tot_slots, 128]
        slot0 = 0
        for phase in range(2):
            dsts = per_core[k]["dsts"][phase]
            nslots = len(dsts)
            valid = dsts >= 0
            out_full[dsts[valid]] = rows[slot0:slot0 + nslots][valid]
            slot0 += nslots
    return out_full


# revision 15
# speedup vs baseline: 1.1451x; 1.0076x over previous
"""Bipartite GCN message-passing kernel for 8 Trainium2 NeuronCores.

Math (reference): rst = deg_in^-1/2 * segsum_dst( (node_f @ W_side) * deg_out^-1/2 [src] )
Refactor (projection is linear, graph strictly bipartite):
    rst[d] = ( sum_{e->d} c_e * f_raw[src_e] ) @ W_side(d),
    c_e = deg_out[src]^-1/2 * deg_in[dst]^-1/2

Division of labor:
  HOST (layout / index math only — no feature arithmetic):
    degree counts, per-core dst dealing, canonical chunk schedule, and a
    bf16 edge-major re-layout of the raw feature rows (M tiles = f[src_e]
    placed at its schedule position) plus compact scatter blocks S holding
    c_e, merged into one stream per window.  This replaces the v1
    device-side dma_gather, whose GPSIMD descriptor generation (~8 ns/edge,
    serial on the Q7s) was a hard 1.6 ms floor.
  DEVICE (all feature FLOPs):
    per window: stream the merged M|S tile sequentially at DMA line rate,
    aggregate PSUM[feat, dst_slot] += M_chunk[128e,128f].T @ S_chunk[128e,cols]
    (bf16 matmuls, fp32 accumulate), then project with W_side (fp32) and
    stream out the [128, slots] feature-major result window by window.

Sharding: dst nodes dealt round-robin (degree-sorted) to 8 cores ->
identical compile-time schedule per core (SPMD), no collectives.
"""
import sys
import os

for _p in ("/opt/trn_rl_repo",):
    if _p not in sys.path and os.path.isdir(_p):
        sys.path.insert(0, _p)

import numpy as np
import ml_dtypes

BF16 = ml_dtypes.bfloat16

N_U = 50000
N_V = 50000
N = N_U + N_V
D = 128
E = 1600000
N_CORES = 8
WIN = 512             # dst slots per PSUM window
P = 128
NBUF = 4              # input stream buffers


# ----------------------------------------------------------------- host layout
def _build_layout(src, dst, cout, cin, u_bf, v_bf):
    """Canonical schedule + per-core merged M|S stream data.

    Returns (wlist, totals, per_core). wlist is the compile-time window
    list in processing order (identical across cores); per_core holds the
    merged ms array + the slot -> global dst id mapping.  Windows are
    processed smallest-first, then descending, 2nd-smallest last, to trim
    the pipeline head and tail; the ms stream is laid out in that order.
    """
    windows = []          # all windows, phase-major creation order
    edges = []            # per phase: dict of per-edge arrays
    per_core_dsts = [[] for _ in range(N_CORES)]
    slot_base = 0

    for phase in range(2):
        if phase == 0:    # dsts are v-nodes, sources u-side
            mask = dst >= N_U
            d_local = dst[mask] - N_U
            s_local = src[mask]
            dst_base = N_U
            src_base = 0
        else:             # dsts are u-nodes, sources v-side
            mask = dst < N_U
            d_local = dst[mask]
            s_local = src[mask] - N_U
            dst_base = 0
            src_base = N_U

        n_dst = N_U
        cnt = np.bincount(d_local, minlength=n_dst)
        order = np.lexsort((np.arange(n_dst), cnt))
        rank = np.empty(n_dst, np.int64)
        rank[order] = np.arange(n_dst)

        spc = n_dst // N_CORES                      # 6250 slots per core
        r = np.arange(n_dst)
        cnt_mat = np.zeros((N_CORES, spc), np.int64)
        cnt_mat[r % N_CORES, r // N_CORES] = cnt[order]
        dst_mat = np.full((N_CORES, spc), -1, np.int64)
        dst_mat[r % N_CORES, r // N_CORES] = order + dst_base
        C = cnt_mat.max(axis=0)                     # canonical slot degrees

        for k in range(N_CORES):
            per_core_dsts[k].append(dst_mat[k])

        # ---- canonical windows + chunk packing (slots may straddle chunks)
        n_win = (spc + WIN - 1) // WIN
        pos_base = np.zeros(spc, np.int64)          # window-local row of slot's 1st edge
        wid0 = len(windows)
        win_nb = np.zeros(n_win, np.int64)
        chunks_col0 = []
        chunks_wscol0 = []
        chunks_win = []
        win_chunk0 = np.zeros(n_win, np.int64)
        pch = 0
        for w in range(n_win):
            s0, s1 = w * WIN, min((w + 1) * WIN, spc)
            Cw = C[s0:s1]
            cum = np.concatenate([[0], np.cumsum(Cw)])
            rows_win = int(cum[-1])
            nb = (rows_win + P - 1) // P
            pos_base[s0:s1] = cum[:-1]
            win_nb[w] = nb
            win_chunk0[w] = pch
            chunks = []
            wsc = 0
            for b in range(nb):
                r0, r1 = b * P, min((b + 1) * P, rows_win)
                first = int(np.searchsorted(cum, r0, side="right")) - 1
                last = int(np.searchsorted(cum, r1, side="left")) - 1
                cols = last - first + 1
                chunks.append({"col0": first, "cols": cols, "wscol0": wsc})
                chunks_col0.append(first)
                chunks_wscol0.append(wsc)
                chunks_win.append(w)
                wsc += cols
            pch += nb
            windows.append({
                "phase": phase,
                "ns": s1 - s0,
                "nb": nb,
                "chunks": chunks,
                "msw": nb * D + wsc,
                "scw": wsc,
                "s0": s0,
            })

        # ---- per-core edge placement (vectorized)
        grp = d_local
        sort_i = np.argsort(grp, kind="stable")
        grp_s = grp[sort_i]
        starts = np.r_[0, np.nonzero(np.diff(grp_s))[0] + 1]
        group_id = np.cumsum(np.r_[0, (np.diff(grp_s) != 0).astype(np.int64)])
        within = np.arange(len(grp_s)) - starts[group_id]
        e_rank = np.empty(len(grp), np.int64)
        e_rank[sort_i] = within

        e_core = rank[d_local] % N_CORES
        e_slot = rank[d_local] // N_CORES
        e_win = e_slot // WIN
        e_lpos = pos_base[e_slot] + e_rank
        e_chunk = win_chunk0[e_win] + e_lpos // P   # phase-local chunk id
        cc0 = np.asarray(chunks_col0, np.int64)
        cw0 = np.asarray(chunks_wscol0, np.int64)
        cwin = np.asarray(chunks_win, np.int64)
        slot_local = e_slot - e_win * WIN
        edges.append({
            "core": e_core,
            "wid": wid0 + cwin[e_chunk],
            "cin_w": e_chunk - win_chunk0[cwin[e_chunk]],   # chunk index in window
            "row": e_lpos % P,
            "wscol": cw0[e_chunk] + slot_local - cc0[e_chunk],
            "src": s_local,
            "val": (cout[s_local + src_base] * cin[d_local + dst_base]
                    ).astype(np.float32),
        })
        slot_base += spc

    # ---- processing order: smallest, then descending, 2nd-smallest last
    by_size = sorted(range(len(windows)), key=lambda i: windows[i]["msw"])
    proc = [by_size[0]] + by_size[1:][::-1]
    ms_base = 0
    oslot = 0
    for wid in proc:
        windows[wid]["ms0"] = ms_base
        ms_base += windows[wid]["msw"]
        windows[wid]["oslot0"] = oslot     # output col base, processing order
        oslot += windows[wid]["ns"]
    wlist = [windows[wid] for wid in proc]

    totals = {
        "tot_ms": ms_base,
        "tot_slots": slot_base,
        "msw_max": max(w["msw"] for w in wlist),
    }

    win_ms0 = np.asarray([w["ms0"] for w in windows], np.int64)
    win_nbg = np.asarray([w["nb"] for w in windows], np.int64)
    feats_by_phase = (u_bf, v_bf)
    per_core = []
    for k in range(N_CORES):
        MS = np.zeros((P, ms_base), BF16)
        for phase in range(2):
            ed = edges[phase]
            m = ed["core"] == k
            wid = ed["wid"][m]
            mcol = win_ms0[wid] + ed["cin_w"][m] * D
            scol = win_ms0[wid] + win_nbg[wid] * D + ed["wscol"][m]
            fcol = mcol[:, None] + np.arange(D)[None, :]
            MS[ed["row"][m][:, None], fcol] = feats_by_phase[phase][ed["src"][m]]
            MS[ed["row"][m], scol] = ed["val"][m].astype(BF16)
        per_core.append({"ms": MS, "dsts": per_core_dsts[k]})
    return wlist, totals, per_core


# ------------------------------------------------------------------ device code
def _build_nc(wlist, totals):
    import concourse.bacc as bacc
    import concourse.bass as bass
    import concourse.mybir as mybir
    from concourse._compat import get_trn_type

    nc = bacc.Bacc(get_trn_type() or "TRN2", target_bir_lowering=False, debug=False)
    f32 = mybir.dt.float32
    bf16 = mybir.dt.bfloat16

    TOTMS = totals["tot_ms"]
    TS = totals["tot_slots"]
    MSWMAX = totals["msw_max"]

    ms_in = nc.dram_tensor("ms", [P, TOTMS], bf16, kind="ExternalInput")
    u_w = nc.dram_tensor("u_w", [D, D], f32, kind="ExternalInput")
    v_w = nc.dram_tensor("v_w", [D, D], f32, kind="ExternalInput")
    out = nc.dram_tensor("out", [P, TS], bf16, kind="ExternalOutput")

    ms_sb = [nc.alloc_sbuf_tensor(f"ms{i}", [P, MSWMAX], bf16) for i in range(NBUF)]
    agg_sb = [nc.alloc_sbuf_tensor(f"agg{i}", [P, WIN], f32) for i in (0, 1)]
    stage_sb = nc.alloc_sbuf_tensor("stage", [P, TS], bf16)
    w_sb = nc.alloc_sbuf_tensor("w", [P, 2, D], f32)

    agg_ps = [nc.alloc_psum_tensor(f"aps{i}", [P, WIN], f32) for i in (0, 1)]
    proj_ps = [nc.alloc_psum_tensor(f"pps{i}", [P, WIN], f32) for i in (0, 1)]

    sem_ld = nc.alloc_semaphore("ld")
    sem_s = [nc.alloc_semaphore(f"ssem{i}") for i in range(NBUF)]
    sem_mm = [nc.alloc_semaphore(f"mmsem{i}") for i in range(NBUF)]
    sem_agg = [nc.alloc_semaphore(f"aggsem{i}") for i in (0, 1)]
    sem_proj = [nc.alloc_semaphore(f"projsem{i}") for i in (0, 1)]
    sem_stage = [nc.alloc_semaphore(f"stsem{i}") for i in (0, 1)]

    NW = len(wlist)
    # cumulative semaphore targets (mm by mod-NBUF class; rest by parity)
    mm_counts = {}
    agg_counts = {}
    agg_counts_prior = {}
    stage_counts = {}
    stage_counts_prior = {}
    mm_c = [0] * NBUF
    agg_c = [0, 0]
    st_c = [0, 0]
    for wi in range(NW):
        b3 = wi % NBUF
        b2 = wi % 2
        mm_c[b3] += 1
        mm_counts[wi] = mm_c[b3]
        agg_counts_prior[wi] = agg_c[b2]
        agg_c[b2] += 1
        agg_counts[wi] = agg_c[b2]
        stage_counts_prior[wi] = st_c[b2]
        st_c[b2] += 1
        stage_counts[wi] = st_c[b2]

    with nc.Block() as block:
        @block.sync
        def _(sy: bass.BassEngine):
            sy.dma_start(w_sb[:, 0, :], u_w[:]).then_inc(sem_ld, 16)
            sy.dma_start(w_sb[:, 1, :], v_w[:]).then_inc(sem_ld, 16)
            for wi, went in enumerate(wlist):
                b3 = wi % NBUF
                if wi >= NBUF:
                    sy.wait_ge(sem_mm[b3], mm_counts[wi - NBUF])
                sy.dma_start(
                    ms_sb[b3][:, :went["msw"]],
                    ms_in[:, went["ms0"]:went["ms0"] + went["msw"]],
                ).then_inc(sem_s[b3], 16)
            sy.wait_ge(sem_ld, 32)

        @block.tensor
        def _(te):
            s_seen = [0] * NBUF

            def proj(j):
                # deferred one window so the vector agg copy overlaps chunks
                wj = wlist[j]
                p2 = j % 2
                if j == 0:
                    te.wait_ge(sem_ld, 32)   # weight matrices resident
                te.wait_ge(sem_agg[p2], agg_counts[j])
                if j >= 2:
                    te.wait_ge(sem_stage[p2], stage_counts_prior[j])
                te.matmul(
                    out=proj_ps[p2][:, :wj["ns"]],
                    lhsT=w_sb[:, wj["phase"], :],
                    rhs=agg_sb[p2][:, :wj["ns"]],
                    start=True, stop=True,
                ).then_inc(sem_proj[p2], 1)

            for wi, went in enumerate(wlist):
                b3 = wi % NBUF
                b2 = wi % 2
                s_seen[b3] += 16
                te.wait_ge(sem_s[b3], s_seen[b3])
                if wi >= 2:
                    te.wait_ge(sem_agg[b2], agg_counts_prior[wi])
                nb = went["nb"]
                soff = nb * D
                for ci, ch in enumerate(went["chunks"]):
                    sc = soff + ch["wscol0"]
                    mm = te.matmul(
                        out=agg_ps[b2][:, ch["col0"]:ch["col0"] + ch["cols"]],
                        lhsT=ms_sb[b3][:, ci * D:(ci + 1) * D],
                        rhs=ms_sb[b3][:, sc:sc + ch["cols"]],
                        start=(ci == 0),
                        stop=(ci == nb - 1),
                    )
                    if ci == nb - 1:
                        mm.then_inc(sem_mm[b3], 1)
                if wi >= 1:
                    proj(wi - 1)
            proj(len(wlist) - 1)

        @block.vector
        def _(ve):
            mm_seen = [0] * NBUF
            for wi, went in enumerate(wlist):
                b3 = wi % NBUF
                b2 = wi % 2
                ns = went["ns"]
                mm_seen[b3] += 1
                ve.wait_ge(sem_mm[b3], mm_seen[b3])
                ve.tensor_copy(out=agg_sb[b2][:, :ns],
                               in_=agg_ps[b2][:, :ns]).then_inc(sem_agg[b2], 1)

        @block.scalar
        def _(sc):
            pr_seen = [0, 0]
            total = 0
            # large output DMA groups, with a tiny final group to trim the tail
            bounds = [9, 15, 20, 24, NW]
            gstart = 0
            for wi, went in enumerate(wlist):
                b2 = wi % 2
                ns = went["ns"]
                pr_seen[b2] += 1
                sc.wait_ge(sem_proj[b2], pr_seen[b2])
                sc.copy(
                    out=stage_sb[:, went["oslot0"]:went["oslot0"] + ns],
                    in_=proj_ps[b2][:, :ns],
                ).then_inc(sem_stage[b2], 1)
                if wi + 1 in bounds:
                    c0 = wlist[gstart]["oslot0"]
                    c1 = went["oslot0"] + ns
                    sc.dma_start(
                        out[:, c0:c1], stage_sb[:, c0:c1]
                    ).then_inc(sem_ld, 16)
                    total += 16
                    gstart = wi + 1
            sc.wait_ge(sem_ld, 32 + total)

    nc.compile()
    return nc


# ---------------------------------------------------------------------- kernel
def kernel(u_f, v_f, u_w, v_w, src, dst):
    from concourse.bass_utils import run_bass_kernel_spmd

    src = np.asarray(src)
    dst = np.asarray(dst)
    u_bf = np.asarray(u_f, np.float32).astype(BF16)
    v_bf = np.asarray(v_f, np.float32).astype(BF16)

    deg_out = np.bincount(src, minlength=N).astype(np.float32)
    deg_in = np.bincount(dst, minlength=N).astype(np.float32)
    cout = np.maximum(deg_out, 1.0) ** -0.5
    cin = np.maximum(deg_in, 1.0) ** -0.5

    wlist, totals, per_core = _build_layout(src, dst, cout, cin, u_bf, v_bf)

    nc = _build_nc(wlist, totals)
    in_maps = []
    for k in range(N_CORES):
        in_maps.append({
            "ms": per_core[k]["ms"],
            "u_w": np.asarray(u_w, np.float32),
            "v_w": np.asarray(v_w, np.float32),
        })
    trace = bool(os.environ.get("KERNEL_TRACE"))
    res = run_bass_kernel_spmd(nc, in_maps, core_ids=list(range(N_CORES)),
                               trace=trace)
    if trace:
        print(f"HW exec time: {res.exec_time_ns} ns")
        kernel.last_profile = res.profile_json

    out_full = np.zeros((N, D), np.float32)
    for k in range(N_CORES):
        fm = res.results[k]["out"]            # [128, tot_slots] bf16
        rows = np.ascontiguousarray(fm.T).astype(np.float32)   # [tot_slots, 128]
        for went in wlist:
            dsts = per_core[k]["dsts"][went["phase"]][went["s0"]:went["s0"] + went["ns"]]
            valid = dsts >= 0
            seg = rows[went["oslot0"]:went["oslot0"] + went["ns"]]
            out_full[dsts[valid]] = seg[valid]
    return out_full
